# revision 8
# baseline (speedup 1.0000x reference)
"""Trainium2 Bass kernel for nn_AdaptiveSample (per-pixel 5x5 sampled softmax
aggregation), distributed over 8 NeuronCores.

Sharding: data-parallel over (batch, H): core i handles batch i//4, rows
[60*(i%4), 60*(i%4)+60). Halo rows are read directly from the full input on
the host (full_io), so no device collectives are needed.

Device layout: partitions = (x-half, row) -> 2*64 = 128 partitions per core
(60 owned rows + 2+2 halo rows per x-half). Free dim = (channel, x) with a
column halo. dx taps become free-dim offsets; dy taps are handled by loading
dy-shifted copies of the inputs straight from DRAM (compute engines cannot
start at arbitrary partitions, DMA can read any DRAM rows). The weighted sum
runs on the VectorEngine in bf16 (2x mode); transcendentals on ScalarEngine.
Even/odd-dx copies keep bf16 operands 4-byte aligned for the 2x DVE mode.

sample_idx is read on the host at call time and the kernel is compiled for
the unique (dy, dx) taps with multiplicities folded into the exp bias
(exp(x + ln m) = m*exp(x)).

guide_weight is all-ones per the problem spec; this is verified at runtime
and a numpy fallback handles the general case.
"""

import os
import sys

for _p in ("/opt/trn_rl_repo", "/root/.axon_site/_ro/trn_rl_repo"):
    if os.path.isdir(_p) and _p not in sys.path:
        sys.path.insert(0, _p)

import numpy as np
import ml_dtypes

import concourse.bass as bass
import concourse.bacc as bacc
import concourse.mybir as mybir
from concourse.tile import TileContext
from concourse.bass_utils import run_bass_kernel_spmd

BF16 = ml_dtypes.bfloat16

K_SIZE = 5
SAMPLE_NUM = 15
DEPTH_MAX = 192.0

B, C, H, W = 2, 32, 240, 320
NCORES = 8
RCH = H * B // NCORES          # 60 owned rows per core
ROWS = RCH + 4                 # 64 rows incl. dy halo
YEXT = ROWS + 4                # 68 DRAM rows (dy-shifted loads need +-2 more)
XH = W // 2                    # 160: x is split in half across partitions
XW = XH + 4                    # 164: x window incl. dx halo
XD = XW + 4                    # 168 DRAM cols (parity-shifted loads)
PW = W + 10                    # padded row width for host prep

_compiled = {}


def _unique_taps(sample_idx):
    """-> sorted tuple of ((dy, dx), mult), dy/dx in [-2, 2]."""
    from collections import Counter
    cnt = Counter()
    for p in np.asarray(sample_idx).tolist():
        cnt[(p // K_SIZE - 2, p % K_SIZE - 2)] += 1
    return tuple(sorted(cnt.items()))


def _tap_src(dx):
    """-> (parity, x-offset) for a 160-wide slice of a parity tile."""
    par = dx & 1
    return par, 2 + dx - par


def _build(taps):
    """Build the per-core Bass program for the given unique taps."""
    U = len(taps)
    f32 = mybir.dt.float32
    bf = mybir.dt.bfloat16
    Alu = mybir.AluOpType
    Act = mybir.ActivationFunctionType

    dys = sorted({dy for (dy, _), _ in taps})
    by_dy = {d: [(j, (dy, dx), m) for j, ((dy, dx), m) in enumerate(taps)
                 if dy == d] for d in dys}

    nc = bacc.Bacc()

    # Register const APs for exp biases ln(mult) (activation bias must be AP).
    need_biases = sorted({float(np.log(m)) for (_, m) in taps if m != 1})
    for val in need_biases:
        t = nc.alloc_sbuf_tensor(f"const-lnm-{val}", [128, 1], f32)
        nc.gpsimd.memset(t.ap(), val)
        nc.const_aps.aps[(f32, val)] = t.ap()
    if need_biases:
        nc.all_engine_barrier()

    d_feat = nc.declare_dram_parameter("feat", [2 * YEXT, C, XD], bf,
                                       isOutput=False)
    d_nrm = nc.declare_dram_parameter("nrm", [2 * YEXT, 3, XD], bf,
                                      isOutput=False)
    d_vld = nc.declare_dram_parameter("vld", [2 * YEXT, XD], bf,
                                      isOutput=False)
    d_nre = nc.declare_dram_parameter("nre", [128, 3, XH], bf, isOutput=False)
    d_out = nc.declare_dram_parameter("out", [128, C, XH], f32, isOutput=True)

    def load_shifted(tile, dram, dy, par, inner):
        """tile[xh*64 + y, ...] <- dram[xh*YEXT + dy+2+y, ..., par:par+XW],
        one fused DMA over both x-halves (rank mismatch is fine: dma_start
        only checks total element count, iteration orders line up)."""
        if inner == 3:
            v = dram[:].rearrange("(a y) c x -> a y c x", a=2)
            src = v[:, dy + 2: dy + 2 + ROWS, :, par:par + XW]
        else:
            v = dram[:].rearrange("(a y) x -> a y x", a=2)
            src = v[:, dy + 2: dy + 2 + ROWS, par:par + XW]
        nc.sync.dma_start(out=tile[:], in_=src)

    with TileContext(nc) as tc:
        with tc.tile_pool(name="p", bufs=1) as pool, \
             tc.tile_pool(name="fp", bufs=3) as fpool, \
             tc.tile_pool(name="wp", bufs=3) as wpool:

            nre = pool.tile([128, 3, XH], bf, tag="nre")
            nc.sync.dma_start(out=nre[:], in_=d_nre[:])

            e_t = pool.tile([128, XH, U], bf, tag="e")

            # ---- weight pipeline, grouped by dy ----
            for dy in dys:
                pars = sorted({_tap_src(dx)[0] for _, (_, dx), _ in by_dy[dy]})
                n_d = {}
                v_d = {}
                for par in pars:
                    n_d[par] = wpool.tile([128, 3, XW], bf, tag=f"nd{par}",
                                          name=f"nrm_dy{dy}_p{par}")
                    load_shifted(n_d[par], d_nrm, dy, par, 3)
                    v_d[par] = wpool.tile([128, XW], bf, tag=f"vd{par}",
                                          name=f"vld_dy{dy}_p{par}")
                    load_shifted(v_d[par], d_vld, dy, par, 1)

                for j, (dy_, dx), mult in by_dy[dy]:
                    par, xo = _tap_src(dx)
                    d3 = pool.tile([128, 3, XH], bf, tag=f"d3_{j % 2}",
                                   name=f"d3_{j}")
                    nc.vector.tensor_tensor(
                        out=d3[:], in0=n_d[par][:, :, xo:xo + XH], in1=nre[:],
                        op=Alu.subtract)
                    d3q = pool.tile([128, 3, XH], f32, tag=f"d3q_{j % 2}",
                                    name=f"d3q_{j}")
                    nc.scalar.activation(out=d3q[:], in_=d3[:], func=Act.Square)
                    dsq = pool.tile([128, XH], f32, tag=f"dsq_{j % 2}",
                                    name=f"dsq_{j}")
                    nc.vector.tensor_reduce(
                        out=dsq[:], in_=d3q[:].rearrange("p c x -> p x c"),
                        axis=mybir.AxisListType.X, op=Alu.add)
                    dif = pool.tile([128, XH], f32, tag=f"dif_{j % 2}",
                                    name=f"dif_{j}")
                    nc.scalar.activation(out=dif[:], in_=dsq[:], func=Act.Sqrt)
                    nw = pool.tile([128, XH], bf, tag=f"nw_{j % 2}",
                                   name=f"nw_{j}")
                    nc.scalar.activation(out=nw[:], in_=dif[:], func=Act.Exp,
                                         scale=-0.5)
                    fwp = pool.tile([128, XH], bf, tag=f"fwp_{j % 2}",
                                    name=f"fwp_{j}")
                    nc.vector.tensor_tensor(
                        out=fwp[:], in0=v_d[par][:, xo:xo + XH], in1=nw[:],
                        op=Alu.mult)
                    # e_j = exp(fw_pre + ln(mult)), written x-major, u inner
                    nc.scalar.activation(out=e_t[:, :, j], in_=fwp[:],
                                         func=Act.Exp, bias=float(np.log(mult)))

            # ---- softmax normalization ----
            z_t = pool.tile([128, XH], f32, tag="z")
            nc.vector.tensor_reduce(out=z_t[:], in_=e_t[:],
                                    axis=mybir.AxisListType.X, op=Alu.add)
            r_t = pool.tile([128, XH], bf, tag="r")
            with nc.allow_low_precision(
                    reason="Z in [15, 41]; bf16 recip well within tolerance"):
                nc.vector.reciprocal(out=r_t[:], in_=z_t[:])
            w_t = pool.tile([128, U, XH], bf, tag="w")
            nc.vector.tensor_tensor(
                out=w_t[:], in0=e_t[:].rearrange("p x u -> p u x"),
                in1=r_t[:][:, None, :].broadcast_to([128, U, XH]),
                op=Alu.mult)

            # ---- MAC: acc = sum_u F_shift_u * w_u (2-group tree) ----
            ngrp = min(2, U)
            accs = [pool.tile([128, C, XH], bf, tag=f"acc{g}", name=f"acc{g}")
                    for g in range(ngrp)]
            tmp = pool.tile([128, C, XH], bf, tag="tmp")
            grp_len = [0] * ngrp
            k = 0
            for dy in dys:
                pars = sorted({_tap_src(dx)[0] for _, (_, dx), _ in by_dy[dy]})
                f_d = {}
                for par in pars:
                    f_d[par] = fpool.tile([128, C, XW], bf, tag=f"fd{par}",
                                          name=f"feat_dy{dy}_p{par}")
                    load_shifted(f_d[par], d_feat, dy, par, 3)
                for j, (dy_, dx), mult in by_dy[dy]:
                    par, xo = _tap_src(dx)
                    fsl = f_d[par][:, :, xo:xo + XH]
                    wb = w_t[:, j, None, :].broadcast_to([128, C, XH])
                    g = k % ngrp
                    if grp_len[g] == 0:
                        nc.vector.tensor_tensor(out=accs[g][:], in0=fsl,
                                                in1=wb, op=Alu.mult)
                    else:
                        nc.vector.tensor_tensor(out=tmp[:], in0=fsl, in1=wb,
                                                op=Alu.mult)
                        nc.vector.tensor_tensor(out=accs[g][:], in0=accs[g][:],
                                                in1=tmp[:], op=Alu.add)
                    grp_len[g] += 1
                    k += 1
            live = [accs[g] for g in range(ngrp) if grp_len[g] > 0]
            while len(live) > 1:
                nxt = []
                for i2 in range(0, len(live) - 1, 2):
                    nc.vector.tensor_tensor(out=live[i2][:], in0=live[i2][:],
                                            in1=live[i2 + 1][:], op=Alu.add)
                    nxt.append(live[i2])
                if len(live) % 2:
                    nxt.append(live[-1])
                live = nxt

            osb = pool.tile([128, C, XH], f32, tag="osb")
            nc.scalar.activation(out=osb[:], in_=live[0][:], func=Act.Copy)
            nc.sync.dma_start(out=d_out[:], in_=osb[:])

    nc.compile()
    return nc


def _prep_core_inputs(i, features, surface_normal, valid_f):
    """Host-side shard prep for core i -> dict of device arrays.

    DRAM row yext maps to image row r0 - 4 + yext (68 rows); DRAM col j maps
    to image col xh*XH + j - 2 (168 cols).
    """
    b = i // 4
    r0 = (i % 4) * RCH
    lo = max(0, r0 - 4)
    hi = min(H, r0 + RCH + 4)
    ylo = lo - (r0 - 4)
    yhi = hi - (r0 - 4)

    fp = np.zeros((C, YEXT, PW), np.float32)
    fp[:, ylo:yhi, 2:2 + W] = features[b, :, lo:hi, :]
    npd = np.zeros((3, YEXT, PW), np.float32)
    npd[:, ylo:yhi, 2:2 + W] = surface_normal[b, :, lo:hi, :]
    vp = np.zeros((YEXT, PW), np.float32)
    vp[ylo:yhi, 2:2 + W] = valid_f[b, lo:hi, :]

    sn_view = surface_normal.reshape(B, H, W, 3)  # raw memory reinterpret
    clo = max(0, r0 - 2)
    chi = min(H, r0 + RCH + 2)
    nre_rows = np.zeros((ROWS, W, 3), np.float32)
    nre_rows[clo - (r0 - 2):chi - (r0 - 2)] = sn_view[b, clo:chi]

    cast = lambda a: np.ascontiguousarray(a).astype(BF16)
    feat = np.concatenate(
        [fp[:, :, xh * XH:xh * XH + XD].transpose(1, 0, 2) for xh in (0, 1)], 0)
    nrm = np.concatenate(
        [npd[:, :, xh * XH:xh * XH + XD].transpose(1, 0, 2) for xh in (0, 1)], 0)
    vld = np.concatenate([vp[:, xh * XH:xh * XH + XD] for xh in (0, 1)], 0)
    nre = np.concatenate(
        [nre_rows[:, xh * XH:(xh + 1) * XH, :].transpose(0, 2, 1)
         for xh in (0, 1)], 0)
    return {"feat": cast(feat), "nrm": cast(nrm), "vld": cast(vld),
            "nre": cast(nre)}


def _run_device(inputs, trace=False):
    features = np.ascontiguousarray(np.asarray(inputs["features"], np.float32))
    surface_normal = np.ascontiguousarray(
        np.asarray(inputs["surface_normal"], np.float32))
    depth = np.asarray(inputs["depth"], np.float32)
    sample_idx = np.asarray(inputs["sample_idx"])

    d = depth[:, 0]
    valid_f = ((d > 0) & (d < DEPTH_MAX)).astype(np.float32)

    taps = _unique_taps(sample_idx)
    if taps not in _compiled:
        _compiled[taps] = _build(taps)
    nc = _compiled[taps]

    in_maps = [_prep_core_inputs(i, features, surface_normal, valid_f)
               for i in range(NCORES)]
    res = run_bass_kernel_spmd(nc, in_maps, list(range(NCORES)), trace=trace)

    out = np.empty((B, C, H, W), np.float32)
    for i in range(NCORES):
        b = i // 4
        r0 = (i % 4) * RCH
        o = np.asarray(res.results[i]["out"], np.float32)  # [128, C, XH]
        for xh in (0, 1):
            sl = o[xh * ROWS + 2: xh * ROWS + 2 + RCH]      # [RCH, C, XH]
            out[b, :, r0:r0 + RCH, xh * XH:(xh + 1) * XH] = sl.transpose(1, 0, 2)
    return out, res


def _reference_numpy(depth, surface_normal, features, guide_weight, sample_idx):
    """Plain-numpy port of the reference (general fallback path)."""
    b, c, h, w = features.shape
    d = depth[:, 0]
    valid = ((d > 0) & (d < DEPTH_MAX)).astype(features.dtype)[:, None]

    def gather(x):
        B_, C_, H_, W_ = x.shape
        xp = np.pad(x, ((0, 0), (0, 0), (2, 2), (2, 2)))
        slabs = []
        for i in range(SAMPLE_NUM):
            p = int(sample_idx[i])
            dy, dx = p // K_SIZE, p % K_SIZE
            slabs.append(xp[:, :, dy:dy + H_, dx:dx + W_])
        return np.stack(slabs, 1).transpose(0, 3, 4, 1, 2)  # [B,H,W,S,C]

    feat_s = gather(features)
    norm_s = gather(surface_normal)
    valid_s = gather(valid)[..., 0]
    center_n = surface_normal.reshape(b, h, w, 3)
    diff = np.sqrt(((norm_s - center_n[:, :, :, None, :]) ** 2).sum(-1))
    normal_w = np.exp(-0.5 * diff)
    guide_s = guide_weight[..., np.asarray(sample_idx)]
    fw = valid_s * normal_w * guide_s
    fw = fw - fw.max(-1, keepdims=True)
    fw = np.exp(fw)
    fw = fw / fw.sum(-1, keepdims=True)
    out = (feat_s * fw[..., None]).sum(3)
    return out.transpose(0, 3, 1, 2).astype(features.dtype)


def kernel(**inputs):
    features = np.asarray(inputs["features"])
    guide = np.asarray(inputs["guide_weight"])
    if not np.all(guide == 1.0):
        # General path (never taken for this problem's spec: fill=ones).
        out = _reference_numpy(
            np.asarray(inputs["depth"], np.float32),
            np.ascontiguousarray(np.asarray(inputs["surface_normal"], np.float32)),
            np.ascontiguousarray(np.asarray(inputs["features"], np.float32)),
            np.asarray(guide, np.float32),
            np.asarray(inputs["sample_idx"]))
        return out, features
    out, _ = _run_device(inputs)
    return out, features


if __name__ == "__main__":
    rng = np.random.default_rng(0)
    inputs = {
        "depth": rng.uniform(0, 200, (B, 1, H, W)).astype(np.float32),
        "surface_normal": rng.standard_normal((B, 3, H, W)).astype(np.float32),
        "features": rng.standard_normal((B, C, H, W)).astype(np.float32),
        "guide_weight": np.ones((B, H, W, 25), np.float32),
        "sample_idx": rng.integers(0, 25, 15).astype(np.int32),
    }
    out, _ = kernel(**inputs)
    exp = _reference_numpy(
        inputs["depth"], inputs["surface_normal"], inputs["features"],
        inputs["guide_weight"], inputs["sample_idx"])
    err = np.linalg.norm(out - exp) / np.linalg.norm(exp)
    print("smoke rel err:", err)


# revision 12
# speedup vs baseline: 2.9983x; 2.9983x over previous
"""Trainium2 Bass kernel for nn_AdaptiveSample (per-pixel 5x5 sampled softmax
aggregation), distributed over 8 NeuronCores.

Sharding: data-parallel over (batch, H): core i handles batch i//4, rows
[60*(i%4), 60*(i%4)+60). Halo rows are read directly from the full input on
the host (full_io), so no device collectives are needed.

Device layout: partitions = (x-half, row) -> 2*64 = 128 partitions per core
(60 owned rows + 2+2 halo rows per x-half). Free dim = (channel, x) with a
column halo. dx taps become free-dim offsets; dy taps are handled by loading
dy-shifted copies of the inputs straight from DRAM (compute engines cannot
start at arbitrary partitions, DMA can read any DRAM rows). The weighted sum
runs on the VectorEngine in bf16 (2x mode); transcendentals on ScalarEngine.
Even/odd-dx copies keep bf16 operands 4-byte aligned for the 2x DVE mode.

sample_idx is read on the host at call time and the kernel is compiled for
the unique (dy, dx) taps with multiplicities folded into the exp bias
(exp(x + ln m) = m*exp(x)).

guide_weight is all-ones per the problem spec; this is verified at runtime
and a numpy fallback handles the general case.
"""

import os
import sys

for _p in ("/opt/trn_rl_repo", "/root/.axon_site/_ro/trn_rl_repo"):
    if os.path.isdir(_p) and _p not in sys.path:
        sys.path.insert(0, _p)

import numpy as np
import ml_dtypes

import concourse.bass as bass
import concourse.bacc as bacc
import concourse.mybir as mybir
from concourse.tile import TileContext
from concourse.bass_utils import run_bass_kernel_spmd

BF16 = ml_dtypes.bfloat16

K_SIZE = 5
SAMPLE_NUM = 15
DEPTH_MAX = 192.0

B, C, H, W = 2, 32, 240, 320
NCORES = 8
RCH = H * B // NCORES          # 60 owned rows per core
ROWS = RCH + 4                 # 64 rows incl. dy halo
YEXT = ROWS + 4                # 68 DRAM rows (dy-shifted loads need +-2 more)
XH = W // 2                    # 160: x is split in half across partitions
XW = XH + 4                    # 164: x window incl. dx halo
XD = XW + 4                    # 168 DRAM cols (parity-shifted loads)
PW = W + 10                    # padded row width for host prep

_compiled = {}


def _unique_taps(sample_idx):
    """-> sorted tuple of ((dy, dx), mult), dy/dx in [-2, 2]."""
    from collections import Counter
    cnt = Counter()
    for p in np.asarray(sample_idx).tolist():
        cnt[(p // K_SIZE - 2, p % K_SIZE - 2)] += 1
    return tuple(sorted(cnt.items()))


def _tap_src(dx):
    """-> (parity, x-offset) for a 160-wide slice of a parity tile."""
    par = dx & 1
    return par, 2 + dx - par


def _variants(taps):
    """Distinct (dy, parity) variant list, in tap (dy-sorted) order."""
    seen = []
    for (dy, dx), _ in taps:
        v = (dy, dx & 1)
        if v not in seen:
            seen.append(v)
    return seen


def _build(taps):
    """Build the per-core Bass program for the given unique taps."""
    U = len(taps)
    f32 = mybir.dt.float32
    bf = mybir.dt.bfloat16
    Alu = mybir.AluOpType
    Act = mybir.ActivationFunctionType

    dys = sorted({dy for (dy, _), _ in taps})
    by_dy = {d: [(j, (dy, dx), m) for j, ((dy, dx), m) in enumerate(taps)
                 if dy == d] for d in dys}
    variants = _variants(taps)
    vidx = {v: i for i, v in enumerate(variants)}
    NV = len(variants)

    nc = bacc.Bacc()

    # Register const APs for exp biases ln(mult) (activation bias must be AP).
    need_biases = sorted({float(np.log(m)) for (_, m) in taps if m != 1})
    for val in need_biases:
        t = nc.alloc_sbuf_tensor(f"const-lnm-{val}", [128, 1], f32)
        nc.gpsimd.memset(t.ap(), val)
        nc.const_aps.aps[(f32, val)] = t.ap()
    if need_biases:
        nc.all_engine_barrier()

    # Each variant is a contiguous [128, ...] DRAM image: the DMA's
    # outermost dim is 128, so packets spray across all 16 DMA engines.
    d_feat = nc.declare_dram_parameter("feat", [NV, 128, C, XW], bf,
                                       isOutput=False)
    d_nrm = nc.declare_dram_parameter("nrm", [NV, 128, 3, XW], bf,
                                      isOutput=False)
    d_vld = nc.declare_dram_parameter("vld", [NV, 128, XW], bf,
                                      isOutput=False)
    d_nre = nc.declare_dram_parameter("nre", [128, 3, XH], bf, isOutput=False)
    d_out = nc.declare_dram_parameter("out", [128, C, XH], f32, isOutput=True)

    dma_eng = [nc.sync, nc.scalar]  # both HWDGE queues

    with TileContext(nc) as tc:
        with tc.tile_pool(name="p", bufs=1) as pool, \
             tc.tile_pool(name="fp", bufs=3) as fpool:

            nre = pool.tile([128, 3, XH], bf, tag="nre")
            nc.sync.dma_start(out=nre[:], in_=d_nre[:])

            # weight-pipeline inputs: load all variants up front
            n_d = {}
            v_d = {}
            for i, v in enumerate(variants):
                n_d[v] = pool.tile([128, 3, XW], bf, tag=f"nd{i}",
                                   name=f"nrm_v{i}")
                dma_eng[i % 2].dma_start(out=n_d[v][:], in_=d_nrm[i])
                v_d[v] = pool.tile([128, XW], bf, tag=f"vd{i}",
                                   name=f"vld_v{i}")
                dma_eng[i % 2].dma_start(out=v_d[v][:], in_=d_vld[i])

            e_t = pool.tile([128, XH, U], bf, tag="e")

            # ---- weight pipeline, phase-ordered so the ScalarEngine runs
            # each activation function in one contiguous batch (the ACT
            # function-table reload costs ~1.3us per switch) ----
            ordered = [(j, (dy, dx), m) for dy in dys
                       for j, (dy, dx), m in by_dy[dy]]

            d3 = [pool.tile([128, 3, XH], bf, tag=f"d3_{j}", name=f"d3_{j}")
                  for j, _, _ in ordered]
            d3q = [pool.tile([128, 3, XH], bf, tag=f"d3q_{j}", name=f"d3q_{j}")
                   for j, _, _ in ordered]
            dsq = [pool.tile([128, XH], f32, tag=f"dsq_{j}", name=f"dsq_{j}")
                   for j, _, _ in ordered]
            dif = [pool.tile([128, XH], f32, tag=f"dif_{j}", name=f"dif_{j}")
                   for j, _, _ in ordered]
            nw = [pool.tile([128, XH], bf, tag=f"nw_{j}", name=f"nw_{j}")
                  for j, _, _ in ordered]
            fwp = [pool.tile([128, XH], bf, tag=f"fwp_{j}", name=f"fwp_{j}")
                   for j, _, _ in ordered]

            for k, (j, (dy, dx), m) in enumerate(ordered):
                par, xo = _tap_src(dx)
                nc.vector.tensor_tensor(
                    out=d3[k][:], in0=n_d[(dy, par)][:, :, xo:xo + XH],
                    in1=nre[:], op=Alu.subtract)
            for k in range(U):
                nc.scalar.activation(out=d3q[k][:], in_=d3[k][:],
                                     func=Act.Square)
            for k in range(U):
                nc.vector.tensor_reduce(
                    out=dsq[k][:], in_=d3q[k][:].rearrange("p c x -> p x c"),
                    axis=mybir.AxisListType.X, op=Alu.add)
            for k in range(U):
                nc.scalar.activation(out=dif[k][:], in_=dsq[k][:],
                                     func=Act.Sqrt)
            for k in range(U):
                nc.scalar.activation(out=nw[k][:], in_=dif[k][:],
                                     func=Act.Exp, scale=-0.5)
            for k, (j, (dy, dx), m) in enumerate(ordered):
                par, xo = _tap_src(dx)
                nc.vector.tensor_tensor(
                    out=fwp[k][:], in0=v_d[(dy, par)][:, xo:xo + XH],
                    in1=nw[k][:], op=Alu.mult)
            for k, (j, (dy, dx), m) in enumerate(ordered):
                # e_j = exp(fw_pre + ln(mult)), written x-major, u inner
                nc.scalar.activation(out=e_t[:, :, j], in_=fwp[k][:],
                                     func=Act.Exp, bias=float(np.log(m)))

            # ---- softmax normalization ----
            z_t = pool.tile([128, XH], f32, tag="z")
            nc.vector.tensor_reduce(out=z_t[:], in_=e_t[:],
                                    axis=mybir.AxisListType.X, op=Alu.add)
            r_t = pool.tile([128, XH], bf, tag="r")
            with nc.allow_low_precision(
                    reason="Z in [15, 41]; bf16 recip well within tolerance"):
                nc.vector.reciprocal(out=r_t[:], in_=z_t[:])
            w_t = pool.tile([128, U, XH], bf, tag="w")
            nc.vector.tensor_tensor(
                out=w_t[:], in0=e_t[:].rearrange("p x u -> p u x"),
                in1=r_t[:][:, None, :].broadcast_to([128, U, XH]),
                op=Alu.mult)

            # ---- MAC: acc = sum_u F_shift_u * w_u (2-group tree) ----
            ngrp = min(2, U)
            accs = [pool.tile([128, C, XH], bf, tag=f"acc{g}", name=f"acc{g}")
                    for g in range(ngrp)]
            tmp = pool.tile([128, C, XH], bf, tag="tmp")
            grp_len = [0] * ngrp
            k = 0
            for dy in dys:
                f_d = {}
                for j, (dy_, dx), mult in by_dy[dy]:
                    par, _ = _tap_src(dx)
                    if par not in f_d:
                        i = vidx[(dy, par)]
                        f_d[par] = fpool.tile([128, C, XW], bf, tag=f"fd{par}",
                                              name=f"feat_v{i}")
                        dma_eng[i % 2].dma_start(out=f_d[par][:], in_=d_feat[i])
                for j, (dy_, dx), mult in by_dy[dy]:
                    par, xo = _tap_src(dx)
                    fsl = f_d[par][:, :, xo:xo + XH]
                    wb = w_t[:, j, None, :].broadcast_to([128, C, XH])
                    g = k % ngrp
                    if grp_len[g] == 0:
                        nc.vector.tensor_tensor(out=accs[g][:], in0=fsl,
                                                in1=wb, op=Alu.mult)
                    else:
                        nc.vector.tensor_tensor(out=tmp[:], in0=fsl, in1=wb,
                                                op=Alu.mult)
                        nc.vector.tensor_tensor(out=accs[g][:], in0=accs[g][:],
                                                in1=tmp[:], op=Alu.add)
                    grp_len[g] += 1
                    k += 1
            live = [accs[g] for g in range(ngrp) if grp_len[g] > 0]
            while len(live) > 1:
                nxt = []
                for i2 in range(0, len(live) - 1, 2):
                    nc.vector.tensor_tensor(out=live[i2][:], in0=live[i2][:],
                                            in1=live[i2 + 1][:], op=Alu.add)
                    nxt.append(live[i2])
                if len(live) % 2:
                    nxt.append(live[-1])
                live = nxt

            osb = pool.tile([128, C, XH], f32, tag="osb")
            nc.scalar.activation(out=osb[:], in_=live[0][:], func=Act.Copy)
            nc.sync.dma_start(out=d_out[:], in_=osb[:])

    nc.compile()
    return nc


def _prep_core_inputs(i, features, surface_normal, valid_f, variants):
    """Host-side shard prep for core i -> dict of device arrays.

    Builds one contiguous [128, ...] image per (dy, parity) variant so each
    device load is a single dense DMA whose outer dim (128) sprays across
    all 16 DMA engines. Padded row yext <-> image row r0 - 4 + yext; padded
    col jj <-> image col jj - 4 (pad 4 left so every variant window is
    in-bounds).
    """
    b = i // 4
    r0 = (i % 4) * RCH
    lo = max(0, r0 - 4)
    hi = min(H, r0 + RCH + 4)
    ylo = lo - (r0 - 4)
    yhi = hi - (r0 - 4)

    fp = np.zeros((YEXT, C, PW), BF16)
    fp[ylo:yhi, :, 4:4 + W] = features[b, :, lo:hi, :].transpose(1, 0, 2)
    npd = np.zeros((YEXT, 3, PW), BF16)
    npd[ylo:yhi, :, 4:4 + W] = surface_normal[b, :, lo:hi, :].transpose(1, 0, 2)
    vp = np.zeros((YEXT, PW), BF16)
    vp[ylo:yhi, 4:4 + W] = valid_f[b, lo:hi, :]

    sn_view = surface_normal.reshape(B, H, W, 3)  # raw memory reinterpret
    clo = max(0, r0 - 2)
    chi = min(H, r0 + RCH + 2)
    nre_rows = np.zeros((ROWS, W, 3), np.float32)
    nre_rows[clo - (r0 - 2):chi - (r0 - 2)] = sn_view[b, clo:chi]
    nre = np.ascontiguousarray(np.concatenate(
        [nre_rows[:, xh * XH:(xh + 1) * XH, :].transpose(0, 2, 1)
         for xh in (0, 1)], 0)).astype(BF16)

    # variant (dy, par): tile[xh*64+y, ..., jj] = img[y + dy + 2, ...,
    # xh*XH + jj + par - 2] -> padded col offset xh*XH + par + 2.
    NV = len(variants)
    feat = np.empty((NV, 128, C, XW), BF16)
    nrm = np.empty((NV, 128, 3, XW), BF16)
    vld = np.empty((NV, 128, XW), BF16)
    for vi, (dy, par) in enumerate(variants):
        ys = dy + 2
        for xh in (0, 1):
            xs = xh * XH + par + 2
            feat[vi, xh * ROWS:(xh + 1) * ROWS] = \
                fp[ys:ys + ROWS, :, xs:xs + XW]
            nrm[vi, xh * ROWS:(xh + 1) * ROWS] = \
                npd[ys:ys + ROWS, :, xs:xs + XW]
            vld[vi, xh * ROWS:(xh + 1) * ROWS] = vp[ys:ys + ROWS, xs:xs + XW]
    return {"feat": feat, "nrm": nrm, "vld": vld, "nre": nre}


def _run_device(inputs, trace=False):
    features = np.ascontiguousarray(np.asarray(inputs["features"], np.float32))
    surface_normal = np.ascontiguousarray(
        np.asarray(inputs["surface_normal"], np.float32))
    depth = np.asarray(inputs["depth"], np.float32)
    sample_idx = np.asarray(inputs["sample_idx"])

    d = depth[:, 0]
    valid_f = ((d > 0) & (d < DEPTH_MAX)).astype(np.float32)

    taps = _unique_taps(sample_idx)
    if taps not in _compiled:
        _compiled[taps] = _build(taps)
    nc = _compiled[taps]

    variants = _variants(taps)
    in_maps = [_prep_core_inputs(i, features, surface_normal, valid_f, variants)
               for i in range(NCORES)]
    res = run_bass_kernel_spmd(nc, in_maps, list(range(NCORES)), trace=trace)

    out = np.empty((B, C, H, W), np.float32)
    for i in range(NCORES):
        b = i // 4
        r0 = (i % 4) * RCH
        o = np.asarray(res.results[i]["out"], np.float32)  # [128, C, XH]
        for xh in (0, 1):
            sl = o[xh * ROWS + 2: xh * ROWS + 2 + RCH]      # [RCH, C, XH]
            out[b, :, r0:r0 + RCH, xh * XH:(xh + 1) * XH] = sl.transpose(1, 0, 2)
    return out, res


def _reference_numpy(depth, surface_normal, features, guide_weight, sample_idx):
    """Plain-numpy port of the reference (general fallback path)."""
    b, c, h, w = features.shape
    d = depth[:, 0]
    valid = ((d > 0) & (d < DEPTH_MAX)).astype(features.dtype)[:, None]

    def gather(x):
        B_, C_, H_, W_ = x.shape
        xp = np.pad(x, ((0, 0), (0, 0), (2, 2), (2, 2)))
        slabs = []
        for i in range(SAMPLE_NUM):
            p = int(sample_idx[i])
            dy, dx = p // K_SIZE, p % K_SIZE
            slabs.append(xp[:, :, dy:dy + H_, dx:dx + W_])
        return np.stack(slabs, 1).transpose(0, 3, 4, 1, 2)  # [B,H,W,S,C]

    feat_s = gather(features)
    norm_s = gather(surface_normal)
    valid_s = gather(valid)[..., 0]
    center_n = surface_normal.reshape(b, h, w, 3)
    diff = np.sqrt(((norm_s - center_n[:, :, :, None, :]) ** 2).sum(-1))
    normal_w = np.exp(-0.5 * diff)
    guide_s = guide_weight[..., np.asarray(sample_idx)]
    fw = valid_s * normal_w * guide_s
    fw = fw - fw.max(-1, keepdims=True)
    fw = np.exp(fw)
    fw = fw / fw.sum(-1, keepdims=True)
    out = (feat_s * fw[..., None]).sum(3)
    return out.transpose(0, 3, 1, 2).astype(features.dtype)


def kernel(**inputs):
    features = np.asarray(inputs["features"])
    guide = np.asarray(inputs["guide_weight"])
    if not np.all(guide == 1.0):
        # General path (never taken for this problem's spec: fill=ones).
        out = _reference_numpy(
            np.asarray(inputs["depth"], np.float32),
            np.ascontiguousarray(np.asarray(inputs["surface_normal"], np.float32)),
            np.ascontiguousarray(np.asarray(inputs["features"], np.float32)),
            np.asarray(guide, np.float32),
            np.asarray(inputs["sample_idx"]))
        return out, features
    out, _ = _run_device(inputs)
    return out, features


if __name__ == "__main__":
    rng = np.random.default_rng(0)
    inputs = {
        "depth": rng.uniform(0, 200, (B, 1, H, W)).astype(np.float32),
        "surface_normal": rng.standard_normal((B, 3, H, W)).astype(np.float32),
        "features": rng.standard_normal((B, C, H, W)).astype(np.float32),
        "guide_weight": np.ones((B, H, W, 25), np.float32),
        "sample_idx": rng.integers(0, 25, 15).astype(np.int32),
    }
    out, _ = kernel(**inputs)
    exp = _reference_numpy(
        inputs["depth"], inputs["surface_normal"], inputs["features"],
        inputs["guide_weight"], inputs["sample_idx"])
    err = np.linalg.norm(out - exp) / np.linalg.norm(exp)
    print("smoke rel err:", err)


# revision 15
# speedup vs baseline: 4.4955x; 1.4994x over previous
"""Trainium2 Bass kernel for nn_AdaptiveSample (per-pixel 5x5 sampled softmax
aggregation), distributed over 8 NeuronCores.

Sharding: data-parallel over (batch, H): core i handles batch i//4, rows
[60*(i%4), 60*(i%4)+60). Halo rows are read directly from the full input on
the host (full_io), so no device collectives are needed.

Device layout: partitions = (x-half, row) -> 2*64 = 128 partitions per core
(60 owned rows + 2+2 halo rows per x-half). Free dim = (channel, x) with a
column halo. dx taps become free-dim offsets; dy taps are handled by loading
dy-shifted copies of the inputs straight from DRAM (compute engines cannot
start at arbitrary partitions, DMA can read any DRAM rows). The weighted sum
runs on the VectorEngine in bf16 (2x mode); transcendentals on ScalarEngine.
Even/odd-dx copies keep bf16 operands 4-byte aligned for the 2x DVE mode.

sample_idx is read on the host at call time and the kernel is compiled for
the unique (dy, dx) taps with multiplicities folded into the exp bias
(exp(x + ln m) = m*exp(x)).

guide_weight is all-ones per the problem spec; this is verified at runtime
and a numpy fallback handles the general case.
"""

import os
import sys

for _p in ("/opt/trn_rl_repo", "/root/.axon_site/_ro/trn_rl_repo"):
    if os.path.isdir(_p) and _p not in sys.path:
        sys.path.insert(0, _p)

import numpy as np
import ml_dtypes

import concourse.bass as bass
import concourse.bacc as bacc
import concourse.mybir as mybir
from concourse.tile import TileContext
from concourse.bass_utils import run_bass_kernel_spmd
from concourse.masks import make_identity

BF16 = ml_dtypes.bfloat16

K_SIZE = 5
SAMPLE_NUM = 15
DEPTH_MAX = 192.0

B, C, H, W = 2, 32, 240, 320
NCORES = 8
RCH = H * B // NCORES          # 60 owned rows per core
ROWS = RCH + 4                 # 64 rows incl. dy halo
YEXT = ROWS + 4                # 68 DRAM rows (dy-shifted loads need +-2 more)
XH = W // 2                    # 160: x is split in half across partitions
XW = XH + 4                    # 164: x window incl. dx halo
XD = XW + 4                    # 168 DRAM cols (parity-shifted loads)
PW = W + 10                    # padded row width for host prep

_compiled = {}


def _unique_taps(sample_idx):
    """-> sorted tuple of ((dy, dx), mult), dy/dx in [-2, 2]."""
    from collections import Counter
    cnt = Counter()
    for p in np.asarray(sample_idx).tolist():
        cnt[(p // K_SIZE - 2, p % K_SIZE - 2)] += 1
    return tuple(sorted(cnt.items()))


def _tap_src(dx):
    """-> (parity, x-offset) for a 160-wide slice of a parity tile."""
    par = dx & 1
    return par, 2 + dx - par


def _variants(taps):
    """Distinct (dy, parity) variant list, in tap (dy-sorted) order."""
    seen = []
    for (dy, dx), _ in taps:
        v = (dy, dx & 1)
        if v not in seen:
            seen.append(v)
    return seen


def _build(taps):
    """Build the per-core Bass program for the given unique taps."""
    U = len(taps)
    f32 = mybir.dt.float32
    bf = mybir.dt.bfloat16
    Alu = mybir.AluOpType
    Act = mybir.ActivationFunctionType

    dys = sorted({dy for (dy, _), _ in taps})
    by_dy = {d: [(j, (dy, dx), m) for j, ((dy, dx), m) in enumerate(taps)
                 if dy == d] for d in dys}
    variants = _variants(taps)
    vidx = {v: i for i, v in enumerate(variants)}
    NV = len(variants)

    nc = bacc.Bacc()

    # Register const APs for exp biases ln(mult) (activation bias must be AP).
    need_biases = sorted({float(np.log(m)) for (_, m) in taps if m != 1})
    for val in need_biases:
        t = nc.alloc_sbuf_tensor(f"const-lnm-{val}", [128, 1], f32)
        nc.gpsimd.memset(t.ap(), val)
        nc.const_aps.aps[(f32, val)] = t.ap()
    if need_biases:
        nc.all_engine_barrier()

    # Each variant is a contiguous [128, ...] DRAM image: the DMA's
    # outermost dim is 128, so packets spray across all 16 DMA engines.
    d_feat = nc.declare_dram_parameter("feat", [NV, 128, C, XW], bf,
                                       isOutput=False)
    d_nrm = nc.declare_dram_parameter("nrm", [NV, 128, 3, XW], bf,
                                      isOutput=False)
    d_vld = nc.declare_dram_parameter("vld", [NV, 128, XW], bf,
                                      isOutput=False)
    d_nre = nc.declare_dram_parameter("nre", [128, 3, XH], bf, isOutput=False)
    d_out = nc.declare_dram_parameter("out", [4, 128, C, XH // 4], f32,
                                      isOutput=True)

    dma_eng = [nc.sync, nc.scalar]  # both HWDGE queues

    with TileContext(nc) as tc:
        with tc.tile_pool(name="p", bufs=1) as pool, \
             tc.tile_pool(name="fp", bufs=1) as fpool, \
             tc.tile_pool(name="ps", bufs=1, space="PSUM") as ppool:

            nre = pool.tile([128, 3, XH], bf, tag="nre")
            nc.sync.dma_start(out=nre[:], in_=d_nre[:])

            # weight-pipeline inputs: load all variants up front
            n_d = {}
            v_d = {}
            for i, v in enumerate(variants):
                n_d[v] = pool.tile([128, 3, XW], bf, tag=f"nd{i}",
                                   name=f"nrm_v{i}")
                dma_eng[i % 2].dma_start(out=n_d[v][:], in_=d_nrm[i])
                v_d[v] = pool.tile([128, XW], bf, tag=f"vd{i}",
                                   name=f"vld_v{i}")
                dma_eng[i % 2].dma_start(out=v_d[v][:], in_=d_vld[i])

            e_t = pool.tile([128, U, XH], bf, tag="e")

            # ---- weight pipeline, phase-ordered so the ScalarEngine runs
            # each activation function in one contiguous batch (the ACT
            # function-table reload costs ~1.3us per switch) ----
            ordered = [(j, (dy, dx), m) for dy in dys
                       for j, (dy, dx), m in by_dy[dy]]

            d3 = [pool.tile([128, 3, XH], bf, tag=f"d3_{j}", name=f"d3_{j}")
                  for j, _, _ in ordered]
            dsq = [pool.tile([128, XH], f32, tag=f"dsq_{j}", name=f"dsq_{j}")
                   for j, _, _ in ordered]
            nw = [pool.tile([128, XH], bf, tag=f"nw_{j}", name=f"nw_{j}")
                  for j, _, _ in ordered]

            for k, (j, (dy, dx), m) in enumerate(ordered):
                par, xo = _tap_src(dx)
                nc.vector.tensor_tensor(
                    out=d3[k][:], in0=n_d[(dy, par)][:, :, xo:xo + XH],
                    in1=nre[:], op=Alu.subtract)
            for k in range(U):
                nc.scalar.activation(out=d3[k][:], in_=d3[k][:],
                                     func=Act.Square)
            for k in range(U):
                nc.vector.tensor_reduce(
                    out=dsq[k][:], in_=d3[k][:].rearrange("p c x -> p x c"),
                    axis=mybir.AxisListType.X, op=Alu.add)
            for k in range(U):
                nc.scalar.activation(out=dsq[k][:], in_=dsq[k][:],
                                     func=Act.Sqrt)
            for k in range(U):
                nc.scalar.activation(out=nw[k][:], in_=dsq[k][:],
                                     func=Act.Exp, scale=-0.5)
            for k, (j, (dy, dx), m) in enumerate(ordered):
                par, xo = _tap_src(dx)
                nc.vector.tensor_tensor(
                    out=nw[k][:], in0=v_d[(dy, par)][:, xo:xo + XH],
                    in1=nw[k][:], op=Alu.mult)
            for k, (j, (dy, dx), m) in enumerate(ordered):
                # e_j = exp(fw_pre + ln(mult)); u-major rows stay dense
                nc.scalar.activation(out=e_t[:, j], in_=nw[k][:],
                                     func=Act.Exp, bias=float(np.log(m)))

            # ---- softmax normalization ----
            z_t = pool.tile([128, XH], f32, tag="z")
            nc.vector.tensor_reduce(out=z_t[:],
                                    in_=e_t[:].rearrange("p u x -> p x u"),
                                    axis=mybir.AxisListType.X, op=Alu.add)
            r_t = pool.tile([128, XH], bf, tag="r")
            with nc.allow_low_precision(
                    reason="Z in [15, 41]; bf16 recip well within tolerance"):
                nc.vector.reciprocal(out=r_t[:], in_=z_t[:])
            w_t = pool.tile([128, U, XH], bf, tag="w")
            nc.vector.tensor_tensor(
                out=w_t[:], in0=e_t[:],
                in1=r_t[:][:, None, :].broadcast_to([128, U, XH]),
                op=Alu.mult)

            # ---- MAC: out = sum_u F_shift_u * w_u ----
            # DVE does the per-pixel broadcast multiplies (bf16, 2x mode);
            # the tap accumulation runs on the idle TensorEngine as
            # identity-weight matmuls accumulating in PSUM (f32).
            ident = pool.tile([128, 128], bf, tag="ident")
            make_identity(nc, ident[:])

            QS = XH // 4                # 40-column PSUM quarters
            QF = C * QS                 # 1280 psum columns per quarter
            f_d = {}
            for i, v in enumerate(variants):
                f_d[v] = fpool.tile([128, C, XW], bf, tag=f"fd{i}",
                                    name=f"feat_v{i}")
                dma_eng[i % 2].dma_start(out=f_d[v][:], in_=d_feat[i])

            for half in range(2):       # two passes of 2 quarters each
                x0 = half * 2 * QS
                tmps = []
                for k, (j, (dy, dx), m) in enumerate(ordered):
                    par, xo = _tap_src(dx)
                    tmp = fpool.tile([128, 2, C, QS], bf, tag="tmp",
                                     name=f"tmp_{half}_{k}", bufs=3)
                    fsl = f_d[(dy, par)][:, :, xo + x0: xo + x0 + 2 * QS]
                    nc.vector.tensor_tensor(
                        out=tmp[:],
                        in0=fsl.rearrange("p c (q x) -> p q c x", q=2),
                        in1=w_t[:, j, x0:x0 + 2 * QS]
                            .rearrange("p (q x) -> p q x", q=2)[:, :, None, :]
                            .broadcast_to([128, 2, C, QS]),
                        op=Alu.mult)
                    tmps.append(tmp)
                pss = [ppool.tile([128, QF], f32, tag=f"ps{q}",
                                  name=f"ps_{half}_{q}") for q in range(2)]
                for k in range(U):
                    tf = tmps[k][:].rearrange("p q c x -> p (q c x)")
                    for q in range(2):
                        for s in range(0, QF, 512):
                            n = min(512, QF - s)
                            nc.tensor.matmul(
                                pss[q][:, s:s + n], ident[:],
                                tf[:, q * QF + s: q * QF + s + n],
                                start=(k == 0), stop=(k == U - 1))
                for q in range(2):
                    oq = fpool.tile([128, QF], f32, tag=f"oq{q}",
                                    name=f"oq_{half}_{q}", bufs=2)
                    nc.scalar.activation(out=oq[:], in_=pss[q][:],
                                         func=Act.Copy)
                    nc.sync.dma_start(out=d_out[half * 2 + q], in_=oq[:])

    nc.compile()
    return nc


def _prep_core_inputs(i, features, surface_normal, valid_f, variants):
    """Host-side shard prep for core i -> dict of device arrays.

    Builds one contiguous [128, ...] image per (dy, parity) variant so each
    device load is a single dense DMA whose outer dim (128) sprays across
    all 16 DMA engines. Padded row yext <-> image row r0 - 4 + yext; padded
    col jj <-> image col jj - 4 (pad 4 left so every variant window is
    in-bounds).
    """
    b = i // 4
    r0 = (i % 4) * RCH
    lo = max(0, r0 - 4)
    hi = min(H, r0 + RCH + 4)
    ylo = lo - (r0 - 4)
    yhi = hi - (r0 - 4)

    fp = np.zeros((YEXT, C, PW), BF16)
    fp[ylo:yhi, :, 4:4 + W] = features[b, :, lo:hi, :].transpose(1, 0, 2)
    npd = np.zeros((YEXT, 3, PW), BF16)
    npd[ylo:yhi, :, 4:4 + W] = surface_normal[b, :, lo:hi, :].transpose(1, 0, 2)
    vp = np.zeros((YEXT, PW), BF16)
    vp[ylo:yhi, 4:4 + W] = valid_f[b, lo:hi, :]

    sn_view = surface_normal.reshape(B, H, W, 3)  # raw memory reinterpret
    clo = max(0, r0 - 2)
    chi = min(H, r0 + RCH + 2)
    nre_rows = np.zeros((ROWS, W, 3), np.float32)
    nre_rows[clo - (r0 - 2):chi - (r0 - 2)] = sn_view[b, clo:chi]
    nre = np.ascontiguousarray(np.concatenate(
        [nre_rows[:, xh * XH:(xh + 1) * XH, :].transpose(0, 2, 1)
         for xh in (0, 1)], 0)).astype(BF16)

    # variant (dy, par): tile[xh*64+y, ..., jj] = img[y + dy + 2, ...,
    # xh*XH + jj + par - 2] -> padded col offset xh*XH + par + 2.
    NV = len(variants)
    feat = np.empty((NV, 128, C, XW), BF16)
    nrm = np.empty((NV, 128, 3, XW), BF16)
    vld = np.empty((NV, 128, XW), BF16)
    for vi, (dy, par) in enumerate(variants):
        ys = dy + 2
        for xh in (0, 1):
            xs = xh * XH + par + 2
            feat[vi, xh * ROWS:(xh + 1) * ROWS] = \
                fp[ys:ys + ROWS, :, xs:xs + XW]
            nrm[vi, xh * ROWS:(xh + 1) * ROWS] = \
                npd[ys:ys + ROWS, :, xs:xs + XW]
            vld[vi, xh * ROWS:(xh + 1) * ROWS] = vp[ys:ys + ROWS, xs:xs + XW]
    return {"feat": feat, "nrm": nrm, "vld": vld, "nre": nre}


def _run_device(inputs, trace=False):
    features = np.ascontiguousarray(np.asarray(inputs["features"], np.float32))
    surface_normal = np.ascontiguousarray(
        np.asarray(inputs["surface_normal"], np.float32))
    depth = np.asarray(inputs["depth"], np.float32)
    sample_idx = np.asarray(inputs["sample_idx"])

    d = depth[:, 0]
    valid_f = ((d > 0) & (d < DEPTH_MAX)).astype(np.float32)

    taps = _unique_taps(sample_idx)
    if taps not in _compiled:
        _compiled[taps] = _build(taps)
    nc = _compiled[taps]

    variants = _variants(taps)
    in_maps = [_prep_core_inputs(i, features, surface_normal, valid_f, variants)
               for i in range(NCORES)]
    res = run_bass_kernel_spmd(nc, in_maps, list(range(NCORES)), trace=trace)

    out = np.empty((B, C, H, W), np.float32)
    for i in range(NCORES):
        b = i // 4
        r0 = (i % 4) * RCH
        o = np.asarray(res.results[i]["out"], np.float32)  # [4,128,C,QS]
        QS = XH // 4
        for q in range(4):
            for xh in (0, 1):
                sl = o[q, xh * ROWS + 2: xh * ROWS + 2 + RCH]  # [RCH, C, QS]
                out[b, :, r0:r0 + RCH,
                    xh * XH + q * QS: xh * XH + (q + 1) * QS] = \
                    sl.transpose(1, 0, 2)
    return out, res


def _reference_numpy(depth, surface_normal, features, guide_weight, sample_idx):
    """Plain-numpy port of the reference (general fallback path)."""
    b, c, h, w = features.shape
    d = depth[:, 0]
    valid = ((d > 0) & (d < DEPTH_MAX)).astype(features.dtype)[:, None]

    def gather(x):
        B_, C_, H_, W_ = x.shape
        xp = np.pad(x, ((0, 0), (0, 0), (2, 2), (2, 2)))
        slabs = []
        for i in range(SAMPLE_NUM):
            p = int(sample_idx[i])
            dy, dx = p // K_SIZE, p % K_SIZE
            slabs.append(xp[:, :, dy:dy + H_, dx:dx + W_])
        return np.stack(slabs, 1).transpose(0, 3, 4, 1, 2)  # [B,H,W,S,C]

    feat_s = gather(features)
    norm_s = gather(surface_normal)
    valid_s = gather(valid)[..., 0]
    center_n = surface_normal.reshape(b, h, w, 3)
    diff = np.sqrt(((norm_s - center_n[:, :, :, None, :]) ** 2).sum(-1))
    normal_w = np.exp(-0.5 * diff)
    guide_s = guide_weight[..., np.asarray(sample_idx)]
    fw = valid_s * normal_w * guide_s
    fw = fw - fw.max(-1, keepdims=True)
    fw = np.exp(fw)
    fw = fw / fw.sum(-1, keepdims=True)
    out = (feat_s * fw[..., None]).sum(3)
    return out.transpose(0, 3, 1, 2).astype(features.dtype)


def kernel(**inputs):
    features = np.asarray(inputs["features"])
    guide = np.asarray(inputs["guide_weight"])
    if not np.all(guide == 1.0):
        # General path (never taken for this problem's spec: fill=ones).
        out = _reference_numpy(
            np.asarray(inputs["depth"], np.float32),
            np.ascontiguousarray(np.asarray(inputs["surface_normal"], np.float32)),
            np.ascontiguousarray(np.asarray(inputs["features"], np.float32)),
            np.asarray(guide, np.float32),
            np.asarray(inputs["sample_idx"]))
        return out, features
    out, _ = _run_device(inputs)
    return out, features


if __name__ == "__main__":
    rng = np.random.default_rng(0)
    inputs = {
        "depth": rng.uniform(0, 200, (B, 1, H, W)).astype(np.float32),
        "surface_normal": rng.standard_normal((B, 3, H, W)).astype(np.float32),
        "features": rng.standard_normal((B, C, H, W)).astype(np.float32),
        "guide_weight": np.ones((B, H, W, 25), np.float32),
        "sample_idx": rng.integers(0, 25, 15).astype(np.int32),
    }
    out, _ = kernel(**inputs)
    exp = _reference_numpy(
        inputs["depth"], inputs["surface_normal"], inputs["features"],
        inputs["guide_weight"], inputs["sample_idx"])
    err = np.linalg.norm(out - exp) / np.linalg.norm(exp)
    print("smoke rel err:", err)
